# revision 22
# baseline (speedup 1.0000x reference)
"""Trainium2 Bass kernel for nn_ConstraintWholePoseScoringModule.

The module scores 3 hardcoded harmonic distance constraints (all on pose 0),
scatter-adds the scores into a [nposes, nblocks, nblocks] block-score matrix,
then sums that matrix per pose -> output [1, nposes].  The scatter + full sum
is algebraically a weighted sum of the constraint scores per pose, so the
kernel never materialises the block-score matrix.

Sharding (per the data-parallel hint): pose dimension split across 8 cores,
2 poses per core, no cross-core communication.  Every core runs the same
program on its shard:

  1. DMA the first 3 block offsets of its local pose 0 (the only pose that
     can host constraints, per the module's constant table).
  2. Two indirect-DMA gathers fetch the constraint endpoint atoms straight
     from DRAM coords (row = block_coord_offset[r] + atom; the +1 atom
     offset of the B endpoints is folded into the DMA element_offset).
  3. diff -> squared-norm (fused square+accumulate) -> sqrt -> (d-4)^2.
  4. A tiny matmul contracts the 2 distance-slot scores against a per-core
     [slot, local_pose] weight table (zeros on cores with no constraints),
     giving that core's [1, 2] per-pose output.

Host side only slices inputs per core and concatenates the [2]-vectors.
"""

import sys

sys.path.insert(0, "/opt/trn_rl_repo")

import numpy as np

NCORES = 8
NPOSES = 16
NBLOCKS = 1024
ATOMS_PER_BLOCK = 16
NATOMS = NBLOCKS * ATOMS_PER_BLOCK  # 16384
PLOC = NPOSES // NCORES  # poses per core = 2
IDEAL = 4.0

# Constant constraint table of the torch module: (pose, (resA, atomA), (resB, atomB)).
_CNSTRS = [
    (0, (0, 0), (1, 1)),
    (0, (1, 0), (2, 1)),
    (0, (0, 0), (1, 1)),
]

# The device program evaluates K=2 distance "slots" on local pose 0 of each
# core: slot k uses atom rows (bco[k] + 0, bco[k+1] + 1).  Each constant
# constraint must map onto one of these slots; its score contributes weight 1
# to its pose.  Verify the constant table matches this structure.
N_SLOTS = 2
for _pose, (_ra, _aa), (_rb, _ab) in _CNSTRS:
    assert _pose % PLOC == 0, "constraints must sit on local pose 0"
    assert (_aa, _ab) == (0, 1) and _rb == _ra + 1 and 0 <= _ra < N_SLOTS


def _slot_weights() -> list[np.ndarray]:
    """Per-core [N_SLOTS, PLOC] weight tables mapping distance-slot scores to
    local poses.  Derived purely from the module's constant constraint table."""
    w = [np.zeros((N_SLOTS, PLOC), np.float32) for _ in range(NCORES)]
    for pose, (ra, _aa), (_rb, _ab) in _CNSTRS:
        w[pose // PLOC][ra, pose % PLOC] += 1.0
    return w


_W_TABLES = _slot_weights()

_CACHE: dict = {}


def _build_bass():
    """Raw Bass program (no Tile): a single semaphore carries the linear
    dependency chain, so every instruction needs at most one sync-wait (the
    HW limit that Tile's auto-scheduling violates for this kernel), and the
    kernel tail is one engine barrier instead of Tile's drain butterfly.

    Semaphore ledger (DMA completions +16, compute +1).  The SWDGE gathers
    need their own semaphore (a SW-DMA sem must start from 0).  Each gather
    gets its own index tile read with a zero AP offset: the HW descriptor
    lowering drops a partition offset on the indirect-offset AP (the
    interpreter honors it, the device does not).
      sem:   idxA dma +16 -> 16   idxB dma +16 -> 32   wt dma +16 -> 48
             sub -> 49   square+accum -> 50   sqrt -> 51   add(-4) -> 52
             square -> 53   matmul -> 54   psum copy -> 55   out dma +16 -> 71
      sem_g: gather A +16 -> 16   gather B +16 -> 32
    """
    import concourse.bass as bass
    import concourse.mybir as mybir

    nc = bass.Bass()
    f32 = mybir.dt.float32
    Square = mybir.ActivationFunctionType.Square

    coords = nc.dram_tensor(
        "coords", [PLOC * NATOMS, 3], f32, kind="ExternalInput"
    )
    bco = nc.dram_tensor(
        "bco", [PLOC * NBLOCKS], mybir.dt.int32, kind="ExternalInput"
    )
    w = nc.dram_tensor("w", [N_SLOTS, PLOC], f32, kind="ExternalInput")
    out_t = nc.dram_tensor("out", [1, PLOC], f32, kind="ExternalOutput")

    with (
        nc.sbuf_tensor([N_SLOTS, 1], mybir.dt.int32) as idxa,
        nc.sbuf_tensor([N_SLOTS, 1], mybir.dt.int32) as idxb,
        nc.sbuf_tensor([N_SLOTS, PLOC], f32) as wt,
        nc.sbuf_tensor([N_SLOTS, 3], f32) as ga,
        nc.sbuf_tensor([N_SLOTS, 3], f32) as gb,
        nc.sbuf_tensor([N_SLOTS, 3], f32) as diff,
        nc.sbuf_tensor([N_SLOTS, 3], f32) as diffsq,
        nc.sbuf_tensor([N_SLOTS, 1], f32) as d2,
        nc.sbuf_tensor([N_SLOTS, 1], f32) as dist,
        nc.sbuf_tensor([N_SLOTS, 1], f32) as dm4,
        nc.sbuf_tensor([N_SLOTS, 1], f32) as score,
        nc.sbuf_tensor([1, PLOC], f32) as osb,
        nc.psum_tensor([1, PLOC], f32) as op,
        nc.semaphore("s") as sem,
        nc.semaphore("sg") as sem_g,
        nc.Block() as block,
    ):

        @block.sync
        def _(sync):
            # block offsets of residues 0..K-1 / 1..K of local pose 0 + weights
            sync.dma_start(out=idxa[:, :], in_=bco[0:N_SLOTS, None]).then_inc(sem, 16)
            sync.dma_start(out=idxb[:, :], in_=bco[1 : N_SLOTS + 1, None]).then_inc(
                sem, 16
            )
            sync.dma_start(out=wt[:, :], in_=w[:, :]).then_inc(sem, 16)
            sync.wait_ge(sem, 55)
            sync.dma_start(out=out_t[:, :], in_=osb[:, :]).then_inc(sem, 16)
            sync.wait_ge(sem, 71)

        @block.gpsimd
        def _(gpsimd):
            # Gather endpoint atoms straight from DRAM: row = bco[r] + atom.
            # A endpoints: residues 0..K-1, atom 0.  B endpoints: residues
            # 1..K, atom 1 (+1 row == +3 elements via element_offset).
            gpsimd.wait_ge(sem, 48)
            gpsimd.indirect_dma_start(
                out=ga[:, :],
                out_offset=None,
                in_=coords[:, :],
                in_offset=bass.IndirectOffsetOnAxis(ap=idxa[:, 0:1], axis=0),
            ).then_inc(sem_g, 16)
            gpsimd.indirect_dma_start(
                out=gb[:, :],
                out_offset=None,
                in_=coords[:, :],
                in_offset=bass.IndirectOffsetOnAxis(ap=idxb[:, 0:1], axis=0),
                element_offset=3,
            ).then_inc(sem_g, 16)

        @block.vector
        def _(vector):
            vector.wait_ge(sem_g, 32)
            vector.tensor_sub(out=diff[:, :], in0=ga[:, :], in1=gb[:, :]).then_inc(
                sem, 1
            )
            vector.wait_ge(sem, 51)
            vector.tensor_scalar_add(
                out=dm4[:, :], in0=dist[:, :], scalar1=-IDEAL
            ).then_inc(sem, 1)

        @block.scalar
        def _(scalar):
            # score_k = (|a_k - b_k| - IDEAL)^2
            scalar.wait_ge(sem, 49)
            scalar.activation(
                out=diffsq[:, :], in_=diff[:, :], func=Square, accum_out=d2[:, :]
            ).then_inc(sem, 1)
            scalar.wait_ge(sem, 50)
            scalar.sqrt(out=dist[:, :], in_=d2[:, :]).then_inc(sem, 1)
            scalar.wait_ge(sem, 52)
            scalar.square(out=score[:, :], in_=dm4[:, :]).then_inc(sem, 1)
            scalar.wait_ge(sem, 54)
            scalar.copy(out=osb[:, :], in_=op[:, :]).then_inc(sem, 1)

        @block.tensor
        def _(tensor):
            # out[p] = sum_k score[k] * w[k, p]
            tensor.wait_ge(sem, 53)
            tensor.matmul(
                out=op[:, :], lhsT=score[:, :], rhs=wt[:, :], start=True, stop=True
            ).then_inc(sem, 1)

    return nc


def _get_nc():
    if "nc" not in _CACHE:
        _CACHE["nc"] = _build_bass()
    return _CACHE["nc"]


def _in_maps(coords: np.ndarray, block_coord_offset: np.ndarray):
    maps = []
    for c in range(NCORES):
        maps.append(
            {
                "coords": np.ascontiguousarray(
                    coords[c * PLOC : (c + 1) * PLOC].reshape(PLOC * NATOMS, 3),
                    dtype=np.float32,
                ),
                "bco": np.ascontiguousarray(
                    block_coord_offset[c * PLOC : (c + 1) * PLOC].reshape(-1),
                    dtype=np.int32,
                ),
                "w": _W_TABLES[c],
            }
        )
    return maps


def run(coords: np.ndarray, block_coord_offset: np.ndarray, **run_kwargs):
    """Run on the 8 NeuronCores; returns (output [1, NPOSES], BassKernelResults)."""
    from concourse.bass_utils import run_bass_kernel_spmd

    nc = _get_nc()
    res = run_bass_kernel_spmd(
        nc,
        _in_maps(np.asarray(coords), np.asarray(block_coord_offset)),
        core_ids=list(range(NCORES)),
        **run_kwargs,
    )
    full = np.zeros((1, NPOSES), np.float32)
    for c in range(NCORES):
        full[0, c * PLOC : (c + 1) * PLOC] = res.results[c]["out"][0]
    return full, res


def kernel(coords: np.ndarray, block_coord_offset: np.ndarray) -> np.ndarray:
    full, _ = run(coords, block_coord_offset)
    return full


# revision 28
# speedup vs baseline: 1.1413x; 1.1413x over previous
"""Trainium2 Bass kernel for nn_ConstraintWholePoseScoringModule.

The module scores 3 hardcoded harmonic distance constraints (all on pose 0),
scatter-adds the scores into a [nposes, nblocks, nblocks] block-score matrix,
then sums that matrix per pose -> output [1, nposes].  The scatter + full sum
is algebraically a weighted sum of the constraint scores per pose, so the
kernel never materialises the block-score matrix.

Sharding (per the data-parallel hint): pose dimension split across 8 cores,
2 poses per core, no cross-core communication.  Every core runs the same
program on its shard:

  1. DMA the first 3 block offsets of its local pose 0 (the only pose that
     can host constraints, per the module's constant table).
  2. Two indirect-DMA gathers fetch the constraint endpoint atoms straight
     from DRAM coords (row = block_coord_offset[r] + atom; the +1 atom
     offset of the B endpoints is folded into the DMA element_offset).
  3. diff -> squared-norm (fused square+accumulate) -> sqrt -> (d-4)^2.
  4. A tiny matmul contracts the 2 distance-slot scores against a per-core
     [slot, local_pose] weight table (zeros on cores with no constraints),
     giving that core's [1, 2] per-pose output.

Host side only slices inputs per core and concatenates the [2]-vectors.
"""

import sys

sys.path.insert(0, "/opt/trn_rl_repo")

import numpy as np

NCORES = 8
NPOSES = 16
NBLOCKS = 1024
ATOMS_PER_BLOCK = 16
NATOMS = NBLOCKS * ATOMS_PER_BLOCK  # 16384
PLOC = NPOSES // NCORES  # poses per core = 2
IDEAL = 4.0

# Constant constraint table of the torch module: (pose, (resA, atomA), (resB, atomB)).
_CNSTRS = [
    (0, (0, 0), (1, 1)),
    (0, (1, 0), (2, 1)),
    (0, (0, 0), (1, 1)),
]

# The device program evaluates K=2 distance "slots" on local pose 0 of each
# core: slot k uses atom rows (bco[k] + 0, bco[k+1] + 1).  Each constant
# constraint must map onto one of these slots; its score contributes weight 1
# to its pose.  Verify the constant table matches this structure.
N_SLOTS = 2
for _pose, (_ra, _aa), (_rb, _ab) in _CNSTRS:
    assert _pose % PLOC == 0, "constraints must sit on local pose 0"
    assert (_aa, _ab) == (0, 1) and _rb == _ra + 1 and 0 <= _ra < N_SLOTS


def _slot_weights() -> list[np.ndarray]:
    """Per-core [N_SLOTS, PLOC] weight tables mapping distance-slot scores to
    local poses.  Derived purely from the module's constant constraint table."""
    w = [np.zeros((N_SLOTS, PLOC), np.float32) for _ in range(NCORES)]
    for pose, (ra, _aa), (_rb, _ab) in _CNSTRS:
        w[pose // PLOC][ra, pose % PLOC] += 1.0
    return w


_W_TABLES = _slot_weights()

_CACHE: dict = {}


def _build_bass():
    """Raw Bass program (no Tile): a single semaphore carries the linear
    dependency chain, so every instruction needs at most one sync-wait (the
    HW limit that Tile's auto-scheduling violates for this kernel), and the
    kernel tail is one engine barrier instead of Tile's drain butterfly.

    Semaphore ledger (DMA completions +16, compute +1).  The SWDGE gathers
    need their own semaphore (a SW-DMA sem must start from 0).  Each gather
    gets its own index tile read with a zero AP offset: the HW descriptor
    lowering drops a partition offset on the indirect-offset AP (the
    interpreter honors it, the device does not).

    Latency tricks (from the NTFF trace of the naive version):
      * the two index DMAs are issued from the two HWDGE engines (SP, ACT) so
        their transfers overlap instead of serializing on one queue; the
        weights DMA gets its own semaphore so the gathers don't wait on it
        (its dependency rejoins via an ACT wait before the Square),
      * a dummy Sqrt on the scalar engine triggers the ~1.3us PWP activation
        table load while the DMAs are still in flight,
      * (d-4)^2 is fused into one ACT Square with a memset -IDEAL bias AP
        (removes the DVE<->ACT ping-pong for the -IDEAL subtract),
      * no_gpsimd_drain skips the ~3us SWDGE drain in the teardown.

      sem:   bias memset -> 1   idxA dma +16 -> 17   idxB dma +16 -> 33
             sub -> 34   square+accum(d2) -> 35   sqrt -> 36
             square(bias) -> 37   matmul -> 38   psum copy -> 39
             out dma +16 -> 55
      sem_w: wt dma +16 -> 16
      sem_g: gather A +16 -> 16   gather B +16 -> 32
    """
    import concourse.bass as bass
    import concourse.mybir as mybir

    nc = bass.Bass()
    f32 = mybir.dt.float32
    Square = mybir.ActivationFunctionType.Square
    Add = mybir.AluOpType.add
    Mult = mybir.AluOpType.mult

    coords = nc.dram_tensor(
        "coords", [PLOC * NATOMS, 3], f32, kind="ExternalInput"
    )
    bco = nc.dram_tensor(
        "bco", [PLOC * NBLOCKS], mybir.dt.int32, kind="ExternalInput"
    )
    w = nc.dram_tensor("w", [N_SLOTS, PLOC], f32, kind="ExternalInput")
    out_t = nc.dram_tensor("out", [1, PLOC], f32, kind="ExternalOutput")

    with (
        nc.sbuf_tensor([N_SLOTS, 1], mybir.dt.int32) as idxa,
        nc.sbuf_tensor([N_SLOTS, 1], mybir.dt.int32) as idxb,
        nc.sbuf_tensor([N_SLOTS, PLOC], f32) as wt,
        nc.sbuf_tensor([N_SLOTS, 3], f32) as ga,
        nc.sbuf_tensor([N_SLOTS, 3], f32) as gb,
        nc.sbuf_tensor([N_SLOTS, 3], f32) as diff,
        nc.sbuf_tensor([N_SLOTS, 3], f32) as diffsq,
        nc.sbuf_tensor([N_SLOTS, 1], f32) as d2,
        nc.sbuf_tensor([N_SLOTS, 1], f32) as dist,
        nc.sbuf_tensor([N_SLOTS, 1], f32) as bias4,
        nc.sbuf_tensor([N_SLOTS, 1], f32) as score,
        nc.sbuf_tensor([N_SLOTS, 1], f32) as scratch,
        nc.sbuf_tensor([1, PLOC], f32) as osb,
        nc.psum_tensor([1, PLOC], f32) as op,
        nc.semaphore("s") as sem,
        nc.semaphore("sw") as sem_w,
        nc.semaphore("sg") as sem_g,
        nc.Block(no_gpsimd_drain=True) as block,
    ):

        @block.sync
        def _(sync):
            # block offsets of residues 0..K-1 of local pose 0
            sync.dma_start(out=idxa[:, :], in_=bco[0:N_SLOTS, None]).then_inc(sem, 16)
            sync.wait_ge(sem, 39)
            sync.dma_start(out=out_t[:, :], in_=osb[:, :]).then_inc(sem, 16)
            sync.wait_ge(sem, 55)

        @block.gpsimd
        def _(gpsimd):
            gpsimd.memset(bias4[:, :], -IDEAL).then_inc(sem, 1)
            # Gather endpoint atoms straight from DRAM: row = bco[r] + atom.
            # A endpoints: residues 0..K-1, atom 0.  B endpoints: residues
            # 1..K, atom 1 (+1 row == +3 elements via element_offset).
            gpsimd.wait_ge(sem, 33)
            gpsimd.indirect_dma_start(
                out=ga[:, :],
                out_offset=None,
                in_=coords[:, :],
                in_offset=bass.IndirectOffsetOnAxis(ap=idxa[:, 0:1], axis=0),
            ).then_inc(sem_g, 16)
            gpsimd.indirect_dma_start(
                out=gb[:, :],
                out_offset=None,
                in_=coords[:, :],
                in_offset=bass.IndirectOffsetOnAxis(ap=idxb[:, 0:1], axis=0),
                element_offset=3,
            ).then_inc(sem_g, 16)

        @block.vector
        def _(vector):
            vector.wait_ge(sem_g, 32)
            vector.tensor_sub(out=diff[:, :], in0=ga[:, :], in1=gb[:, :]).then_inc(
                sem, 1
            )

        @block.scalar
        def _(scalar):
            # idxB + weights on ACT's HWDGE queue (parallel to SP's idxA), then
            # warm the PWP activation table while the DMAs are in flight
            # (scale=0.0 so the dummy never reads the uninitialized scratch)
            scalar.dma_start(out=idxb[:, :], in_=bco[1 : N_SLOTS + 1, None]).then_inc(
                sem, 16
            )
            scalar.dma_start(out=wt[:, :], in_=w[:, :]).then_inc(sem_w, 16)
            scalar.activation(
                out=scratch[:, :],
                in_=scratch[:, :],
                func=mybir.ActivationFunctionType.Sqrt,
                scale=0.0,
            )
            # score_k = (sqrt(d2_k) - IDEAL)^2
            scalar.wait_ge(sem, 34)
            scalar.activation(
                out=diffsq[:, :], in_=diff[:, :], func=Square, accum_out=d2[:, :]
            ).then_inc(sem, 1)
            scalar.wait_ge(sem, 35)
            scalar.sqrt(out=dist[:, :], in_=d2[:, :]).then_inc(sem, 1)
            scalar.wait_ge(sem_w, 16)
            scalar.wait_ge(sem, 36)
            scalar.activation(
                out=score[:, :], in_=dist[:, :], func=Square, bias=bias4[:, 0:1]
            ).then_inc(sem, 1)
            scalar.wait_ge(sem, 38)
            scalar.copy(out=osb[:, :], in_=op[:, :]).then_inc(sem, 1)

        @block.tensor
        def _(tensor):
            # out[p] = sum_k score[k] * w[k, p]  (wt covered transitively: the
            # Square is preceded by the sem_w wait on the ACT engine)
            tensor.wait_ge(sem, 37)
            tensor.matmul(
                out=op[:, :], lhsT=score[:, :], rhs=wt[:, :], start=True, stop=True
            ).then_inc(sem, 1)

    return nc


def _get_nc():
    if "nc" not in _CACHE:
        _CACHE["nc"] = _build_bass()
    return _CACHE["nc"]


def _in_maps(coords: np.ndarray, block_coord_offset: np.ndarray):
    maps = []
    for c in range(NCORES):
        maps.append(
            {
                "coords": np.ascontiguousarray(
                    coords[c * PLOC : (c + 1) * PLOC].reshape(PLOC * NATOMS, 3),
                    dtype=np.float32,
                ),
                "bco": np.ascontiguousarray(
                    block_coord_offset[c * PLOC : (c + 1) * PLOC].reshape(-1),
                    dtype=np.int32,
                ),
                "w": _W_TABLES[c],
            }
        )
    return maps


def run(coords: np.ndarray, block_coord_offset: np.ndarray, **run_kwargs):
    """Run on the 8 NeuronCores; returns (output [1, NPOSES], BassKernelResults)."""
    from concourse.bass_utils import run_bass_kernel_spmd

    nc = _get_nc()
    res = run_bass_kernel_spmd(
        nc,
        _in_maps(np.asarray(coords), np.asarray(block_coord_offset)),
        core_ids=list(range(NCORES)),
        **run_kwargs,
    )
    full = np.zeros((1, NPOSES), np.float32)
    for c in range(NCORES):
        full[0, c * PLOC : (c + 1) * PLOC] = res.results[c]["out"][0]
    return full, res


def kernel(coords: np.ndarray, block_coord_offset: np.ndarray) -> np.ndarray:
    full, _ = run(coords, block_coord_offset)
    return full


# revision 29
# speedup vs baseline: 1.1528x; 1.0101x over previous
"""Trainium2 Bass kernel for nn_ConstraintWholePoseScoringModule.

The module scores 3 hardcoded harmonic distance constraints (all on pose 0),
scatter-adds the scores into a [nposes, nblocks, nblocks] block-score matrix,
then sums that matrix per pose -> output [1, nposes].  The scatter + full sum
is algebraically a weighted sum of the constraint scores per pose, so the
kernel never materialises the block-score matrix.

Sharding (per the data-parallel hint): pose dimension split across 8 cores,
2 poses per core, no cross-core communication.  Every core runs the same
program on its shard:

  1. DMA the first 3 block offsets of its local pose 0 (the only pose that
     can host constraints, per the module's constant table).
  2. Two indirect-DMA gathers fetch the constraint endpoint atoms straight
     from DRAM coords (row = block_coord_offset[r] + atom; the +1 atom
     offset of the B endpoints is folded into the DMA element_offset).
  3. diff -> squared-norm (fused square+accumulate) -> sqrt -> (d-4)^2.
  4. A tiny matmul contracts the 2 distance-slot scores against a per-core
     [slot, local_pose] weight table (zeros on cores with no constraints),
     giving that core's [1, 2] per-pose output.

Host side only slices inputs per core and concatenates the [2]-vectors.
"""

import sys

sys.path.insert(0, "/opt/trn_rl_repo")

import numpy as np

NCORES = 8
NPOSES = 16
NBLOCKS = 1024
ATOMS_PER_BLOCK = 16
NATOMS = NBLOCKS * ATOMS_PER_BLOCK  # 16384
PLOC = NPOSES // NCORES  # poses per core = 2
IDEAL = 4.0

# Constant constraint table of the torch module: (pose, (resA, atomA), (resB, atomB)).
_CNSTRS = [
    (0, (0, 0), (1, 1)),
    (0, (1, 0), (2, 1)),
    (0, (0, 0), (1, 1)),
]

# The device program evaluates K=2 distance "slots" on local pose 0 of each
# core: slot k uses atom rows (bco[k] + 0, bco[k+1] + 1).  Each constant
# constraint must map onto one of these slots; its score contributes weight 1
# to its pose.  Verify the constant table matches this structure.
N_SLOTS = 2
for _pose, (_ra, _aa), (_rb, _ab) in _CNSTRS:
    assert _pose % PLOC == 0, "constraints must sit on local pose 0"
    assert (_aa, _ab) == (0, 1) and _rb == _ra + 1 and 0 <= _ra < N_SLOTS


def _slot_weights() -> list[np.ndarray]:
    """Per-core [N_SLOTS, PLOC] weight tables mapping distance-slot scores to
    local poses.  Derived purely from the module's constant constraint table."""
    w = [np.zeros((N_SLOTS, PLOC), np.float32) for _ in range(NCORES)]
    for pose, (ra, _aa), (_rb, _ab) in _CNSTRS:
        w[pose // PLOC][ra, pose % PLOC] += 1.0
    return w


_W_TABLES = _slot_weights()

_CACHE: dict = {}


def _build_bass():
    """Raw Bass program (no Tile): a single semaphore carries the linear
    dependency chain, so every instruction needs at most one sync-wait (the
    HW limit that Tile's auto-scheduling violates for this kernel), and the
    kernel tail is one engine barrier instead of Tile's drain butterfly.

    Semaphore ledger (DMA completions +16, compute +1).  The SWDGE gathers
    need their own semaphore (a SW-DMA sem must start from 0).  Each gather
    gets its own index tile read with a zero AP offset: the HW descriptor
    lowering drops a partition offset on the indirect-offset AP (the
    interpreter honors it, the device does not).

    Latency tricks (from NTFF traces of earlier versions):
      * ONE index DMA loads a [K, 2] tile (partition k = bco[k], bco[k+1]);
        gather A reads column 0, gather B column 1 (free-dim offset on the
        indirect-offset AP) -- one HWDGE transfer on SP's queue and the
        gathers wait on nothing else,
      * a dummy Sqrt issued first on the scalar engine triggers the ~1.3us
        PWP activation table load while the DMAs are still in flight; the
        weights DMA follows on ACT's queue with its own semaphore (the PE
        dependency rejoins via a DVE wait before the score multiply),
      * all arithmetic except sqrt runs on the DVE (~150ns/op vs ~570ns/op
        on ACT), including the PSUM->SBUF copy of the matmul result,
      * no_gpsimd_drain skips the ~3us SWDGE drain in the teardown.

      sem:   idx dma +16 -> 16   sub -> 17   mul -> 18   reduce(d2) -> 19
             sqrt -> 20   add(-IDEAL) -> 21   mul(score) -> 22   matmul -> 23
             psum copy -> 24   out dma +16 -> 40
      sem_w: wt dma +16 -> 16
      sem_g: gather A +16 -> 16   gather B +16 -> 32
    """
    import concourse.bass as bass
    import concourse.mybir as mybir

    nc = bass.Bass()
    f32 = mybir.dt.float32

    coords = nc.dram_tensor(
        "coords", [PLOC * NATOMS, 3], f32, kind="ExternalInput"
    )
    bco = nc.dram_tensor(
        "bco", [PLOC * NBLOCKS], mybir.dt.int32, kind="ExternalInput"
    )
    w = nc.dram_tensor("w", [N_SLOTS, PLOC], f32, kind="ExternalInput")
    out_t = nc.dram_tensor("out", [1, PLOC], f32, kind="ExternalOutput")

    with (
        nc.sbuf_tensor([N_SLOTS, 2], mybir.dt.int32) as idx,
        nc.sbuf_tensor([N_SLOTS, PLOC], f32) as wt,
        nc.sbuf_tensor([N_SLOTS, 3], f32) as ga,
        nc.sbuf_tensor([N_SLOTS, 3], f32) as gb,
        nc.sbuf_tensor([N_SLOTS, 3], f32) as diff,
        nc.sbuf_tensor([N_SLOTS, 3], f32) as diffsq,
        nc.sbuf_tensor([N_SLOTS, 1], f32) as d2,
        nc.sbuf_tensor([N_SLOTS, 1], f32) as dist,
        nc.sbuf_tensor([N_SLOTS, 1], f32) as dm4,
        nc.sbuf_tensor([N_SLOTS, 1], f32) as score,
        nc.sbuf_tensor([N_SLOTS, 1], f32) as scratch,
        nc.sbuf_tensor([1, PLOC], f32) as osb,
        nc.psum_tensor([1, PLOC], f32) as op,
        nc.semaphore("s") as sem,
        nc.semaphore("sw") as sem_w,
        nc.semaphore("sg") as sem_g,
        nc.Block(no_gpsimd_drain=True) as block,
    ):

        @block.sync
        def _(sync):
            # idx[k] = (bco[k], bco[k+1]): one overlapped-AP transfer
            sync.dma_start(
                out=idx[:, :],
                in_=bass.AP(bco, 0, [[1, N_SLOTS], [1, 2]]),
            ).then_inc(sem, 16)
            sync.wait_ge(sem, 24)
            sync.dma_start(out=out_t[:, :], in_=osb[:, :]).then_inc(sem, 16)
            sync.wait_ge(sem, 40)

        @block.gpsimd
        def _(gpsimd):
            # Gather endpoint atoms straight from DRAM: row = bco[r] + atom.
            # A endpoints: residues 0..K-1, atom 0.  B endpoints: residues
            # 1..K, atom 1 (+1 row == +3 elements via element_offset).
            gpsimd.wait_ge(sem, 16)
            gpsimd.indirect_dma_start(
                out=ga[:, :],
                out_offset=None,
                in_=coords[:, :],
                in_offset=bass.IndirectOffsetOnAxis(ap=idx[:, 0:1], axis=0),
            ).then_inc(sem_g, 16)
            gpsimd.indirect_dma_start(
                out=gb[:, :],
                out_offset=None,
                in_=coords[:, :],
                in_offset=bass.IndirectOffsetOnAxis(ap=idx[:, 1:2], axis=0),
                element_offset=3,
            ).then_inc(sem_g, 16)

        @block.vector
        def _(vector):
            # d2_k = |a_k - b_k|^2
            vector.wait_ge(sem_g, 32)
            vector.tensor_sub(out=diff[:, :], in0=ga[:, :], in1=gb[:, :]).then_inc(
                sem, 1
            )
            vector.wait_ge(sem, 17)
            vector.tensor_mul(
                out=diffsq[:, :], in0=diff[:, :], in1=diff[:, :]
            ).then_inc(sem, 1)
            vector.wait_ge(sem, 18)
            vector.reduce_sum(
                out=d2[:, :], in_=diffsq[:, :], axis=mybir.AxisListType.X
            ).then_inc(sem, 1)
            # score_k = (dist_k - IDEAL)^2
            vector.wait_ge(sem, 20)
            vector.tensor_scalar_add(
                out=dm4[:, :], in0=dist[:, :], scalar1=-IDEAL
            ).then_inc(sem, 1)
            vector.wait_ge(sem_w, 16)
            vector.wait_ge(sem, 21)
            vector.tensor_mul(out=score[:, :], in0=dm4[:, :], in1=dm4[:, :]).then_inc(
                sem, 1
            )
            vector.wait_ge(sem, 23)
            vector.tensor_copy(out=osb[:, :], in_=op[:, :]).then_inc(sem, 1)

        @block.scalar
        def _(scalar):
            # warm the PWP activation table immediately (scale=0.0 so the
            # dummy never reads the uninitialized scratch), then the weights
            scalar.activation(
                out=scratch[:, :],
                in_=scratch[:, :],
                func=mybir.ActivationFunctionType.Sqrt,
                scale=0.0,
            )
            scalar.dma_start(out=wt[:, :], in_=w[:, :]).then_inc(sem_w, 16)
            scalar.wait_ge(sem, 19)
            scalar.sqrt(out=dist[:, :], in_=d2[:, :]).then_inc(sem, 1)

        @block.tensor
        def _(tensor):
            # out[p] = sum_k score[k] * w[k, p]  (wt covered transitively: the
            # score multiply is preceded by the sem_w wait on the DVE)
            tensor.wait_ge(sem, 22)
            tensor.matmul(
                out=op[:, :], lhsT=score[:, :], rhs=wt[:, :], start=True, stop=True
            ).then_inc(sem, 1)

    return nc


def _get_nc():
    if "nc" not in _CACHE:
        _CACHE["nc"] = _build_bass()
    return _CACHE["nc"]


def _in_maps(coords: np.ndarray, block_coord_offset: np.ndarray):
    maps = []
    for c in range(NCORES):
        maps.append(
            {
                "coords": np.ascontiguousarray(
                    coords[c * PLOC : (c + 1) * PLOC].reshape(PLOC * NATOMS, 3),
                    dtype=np.float32,
                ),
                "bco": np.ascontiguousarray(
                    block_coord_offset[c * PLOC : (c + 1) * PLOC].reshape(-1),
                    dtype=np.int32,
                ),
                "w": _W_TABLES[c],
            }
        )
    return maps


def run(coords: np.ndarray, block_coord_offset: np.ndarray, **run_kwargs):
    """Run on the 8 NeuronCores; returns (output [1, NPOSES], BassKernelResults)."""
    from concourse.bass_utils import run_bass_kernel_spmd

    nc = _get_nc()
    res = run_bass_kernel_spmd(
        nc,
        _in_maps(np.asarray(coords), np.asarray(block_coord_offset)),
        core_ids=list(range(NCORES)),
        **run_kwargs,
    )
    full = np.zeros((1, NPOSES), np.float32)
    for c in range(NCORES):
        full[0, c * PLOC : (c + 1) * PLOC] = res.results[c]["out"][0]
    return full, res


def kernel(coords: np.ndarray, block_coord_offset: np.ndarray) -> np.ndarray:
    full, _ = run(coords, block_coord_offset)
    return full
